# revision 1
# baseline (speedup 1.0000x reference)
"""KimiLinear KDA decode step — Trainium2 Bass kernel (8 NeuronCores).

Problem: B=128 decode batch, HK=HV=32 heads, D=128 head dim, K=4 causal conv.
  1. per-channel causal conv1d update + silu over mixed_qkv (12288 channels)
  2. split q/k/v, l2norm(q)*D^-0.5, l2norm(k)
  3. fused KDA gate g = -exp(A_log)*softplus(forget_gate + dt_bias), b=sigmoid(beta)
  4. gated delta-rule readout:
       S' = S * exp(g);  kv = k @ S';  delta = (v - kv)*b
       o  = q @ (S' + k (x) delta) = q @ S' + (q.k) * delta
     The updated state is never materialized: only two mat-vecs against S plus
     the (q.k) rank-1 correction are needed.

Sharding: data-parallel over batch — 16 batches per core; each core handles all
32 heads of its batch slice with zero cross-core communication (matches the
sharding hint: states shard with batch).

Device data layout ("layout A"): all per-token tensors live in SBUF as
[128 partitions = d (head dim), free = h*16 + b] so that
  - the conv is purely elementwise (channel c = sec*4096 + h*128 + d maps to
    partition d, free (sec,h,b)),
  - q/k/v vectors are matmul-ready on the contraction (d) partition axis,
  - per-(b,h) scalars (norms, q.k) are produced/broadcast with tiny
    ones-matmuls on the otherwise idle TensorE.
Host-side staging only reshapes/transposes/replicates activations (layout
choice at upload time); the model weights (conv_weights / A_log / dt_bias) are
additionally pre-folded (-exp(A_log)) per standard inference weight prep.
All arithmetic on activations happens on device in fp32.

Per core HBM traffic ~37 MB (dominated by the 33.5 MB ssm_state slice) — the
kernel is memory-bound; the 512 per-(b,h) fp32 matmuls (stationary = S[b,h],
moving = [k_gated | q_gated] 2 columns) hide under the DMA stream.
"""

import numpy as np

import concourse.bass as bass
import concourse.bacc as bacc
import concourse.mybir as mybir
from concourse.tile import TileContext
from concourse.bass_utils import run_bass_kernel_spmd

F32 = mybir.dt.float32
AF = mybir.ActivationFunctionType
OP = mybir.AluOpType

NCORES = 8
B, HK, HV, D, CK = 128, 32, 32, 128, 4
SEC = 3                      # q | k | v channel sections of 32 heads each
BC = B // NCORES             # batches per core = 16
NHB = HV * BC                # free columns per section = 512
QKV = (2 * HK + HV) * D      # 12288
GW = 8                       # batches per psum output group (2 groups)

_CACHE = {}


def _build_nc():
    # Bacc (not raw Bass): its compile() splits multi-sem waits into event
    # semaphores — TRN2 instructions carry at most one wait.
    nc = bacc.Bacc("TRN2", target_bir_lowering=False, debug=False)
    xq = nc.declare_dram_parameter("xq", [D, SEC * NHB], F32, isOutput=False)
    cst = nc.declare_dram_parameter("cst", [D, 3 * SEC * NHB], F32, isOutput=False)
    wrep = nc.declare_dram_parameter("wrep", [D, 4 * SEC * NHB], F32, isOutput=False)
    # aux = [forget_gate | dt_bias | -exp(A_log) | beta] side by side
    aux = nc.declare_dram_parameter("aux", [D, 4 * NHB], F32, isOutput=False)
    F16 = mybir.dt.float16
    # ssm shipped as an fp16 hi/lo pair (value-exact to ~21 mantissa bits,
    # same 4 B/elem of HBM traffic as fp32): the fp16 stationary gets the PE
    # fast-weight-load path that fp32 self-loading matmuls cannot use.
    ssm_hi = nc.declare_dram_parameter("ssm_hi", [BC, HV, D, D], F16,
                                       isOutput=False)
    ssm_lo = nc.declare_dram_parameter("ssm_lo", [BC, HV, D, D], F16,
                                       isOutput=False)
    o_out = nc.declare_dram_parameter("o_out", [D, NHB], F32, isOutput=True)

    S3 = SEC * NHB  # 1536

    with TileContext(nc) as tc:
        with (
            tc.tile_pool(name="const", bufs=1) as const,
            tc.tile_pool(name="work", bufs=1) as work,
            tc.tile_pool(name="spool", bufs=2) as spool,
            tc.tile_pool(name="psr", bufs=1, space="PSUM") as psr,
            tc.tile_pool(name="psb", bufs=1, space="PSUM") as psb,
            tc.tile_pool(name="pso", bufs=1, space="PSUM") as pso,
        ):
            # ---- input staging --------------------------------------------
            t_cst = const.tile([D, 3 * S3], F32)
            nc.sync.dma_start(t_cst[:], cst[:])
            t_xq = const.tile([D, S3], F32)
            nc.sync.dma_start(t_xq[:], xq[:])
            t_w = const.tile([D, 4 * S3], F32)
            nc.sync.dma_start(t_w[:], wrep[:])
            t_aux = const.tile([D, 4 * NHB], F32)
            nc.scalar.dma_start(t_aux[:], aux[:])
            t_fg = t_aux[:, 0:NHB]
            t_dtb = t_aux[:, NHB:2 * NHB]
            t_nega = t_aux[:, 2 * NHB:3 * NHB]
            t_beta = t_aux[:, 3 * NHB:4 * NHB]

            ones_c = const.tile([D, 1], F32)
            nc.vector.memset(ones_c[:], 1.0)
            ones_r = const.tile([1, D], F32)
            nc.vector.memset(ones_r[:], 1.0)
            ones_rs = const.tile([1, D], F32)
            nc.vector.memset(ones_rs[:], float(D) ** -0.5)

            # ---- causal conv1d single-step + silu -------------------------
            acc = work.tile([D, S3], F32)
            tmp = work.tile([D, S3], F32)
            nc.vector.tensor_tensor(acc[:], t_cst[:, 0:S3], t_w[:, 0:S3], OP.mult)
            for j in (1, 2):
                nc.vector.tensor_tensor(
                    tmp[:], t_cst[:, j * S3:(j + 1) * S3],
                    t_w[:, j * S3:(j + 1) * S3], OP.mult)
                nc.vector.tensor_tensor(acc[:], acc[:], tmp[:], OP.add)
            nc.vector.tensor_tensor(tmp[:], t_xq[:], t_w[:, 3 * S3:4 * S3], OP.mult)
            nc.vector.tensor_tensor(acc[:], acc[:], tmp[:], OP.add)
            x = work.tile([D, S3], F32)
            nc.scalar.activation(x[:], acc[:], AF.Silu)
            q = x[:, 0:NHB]
            k = x[:, NHB:2 * NHB]
            v = x[:, 2 * NHB:3 * NHB]

            # ---- l2 norms (partition reduce via ones-matmul) --------------
            sq = work.tile([D, 2 * NHB], F32)
            nc.vector.tensor_tensor(sq[:, 0:NHB], q, q, OP.mult)
            nc.vector.tensor_tensor(sq[:, NHB:2 * NHB], k, k, OP.mult)
            nrow = psr.tile([1, 2 * NHB], F32)
            nc.tensor.matmul(nrow[:, 0:NHB], ones_c[:], sq[:, 0:NHB],
                             start=True, stop=True)
            nc.tensor.matmul(nrow[:, NHB:2 * NHB], ones_c[:], sq[:, NHB:2 * NHB],
                             start=True, stop=True)
            neps = work.tile([1, 2 * NHB], F32)
            nc.vector.tensor_scalar_add(neps[:], nrow[:], 1e-6)
            rrow = work.tile([1, 2 * NHB], F32)
            nc.vector.reciprocal(rrow[:], neps[:])
            srow = work.tile([1, 2 * NHB], F32)
            nc.scalar.activation(srow[:], rrow[:], AF.Sqrt)  # rsqrt = sqrt(1/x)

            # broadcast 1/||q||*D^-0.5 and 1/||k|| along partitions
            rb = psb.tile([D, 2 * NHB], F32)
            nc.tensor.matmul(rb[:, 0:NHB], ones_rs[:], srow[:, 0:NHB],
                             start=True, stop=True)
            nc.tensor.matmul(rb[:, NHB:2 * NHB], ones_r[:], srow[:, NHB:2 * NHB],
                             start=True, stop=True)
            qh = work.tile([D, NHB], F32)
            nc.vector.tensor_tensor(qh[:], q, rb[:, 0:NHB], OP.mult)
            kh = work.tile([D, NHB], F32)
            nc.vector.tensor_tensor(kh[:], k, rb[:, NHB:2 * NHB], OP.mult)

            # ---- KDA gate: eg = exp(-exp(A_log)*softplus(fg+dt_bias)) -----
            # no softplus ACT table on this compiler: use the numerically
            # stable split softplus(x) = relu(x) + ln(1 + exp(-|x|)) so exp/ln
            # share one table with the final exp.
            g1 = work.tile([D, NHB], F32)
            nc.vector.tensor_tensor(g1[:], t_fg[:], t_dtb[:], OP.add)
            ga = work.tile([D, NHB], F32)
            nc.scalar.activation(ga[:], g1[:], AF.Abs)
            nc.scalar.activation(ga[:], ga[:], AF.Exp, scale=-1.0)
            nc.scalar.activation(ga[:], ga[:], AF.Ln, bias=1.0)
            gr = work.tile([D, NHB], F32)
            nc.vector.tensor_scalar_max(gr[:], g1[:], 0.0)
            sp = work.tile([D, NHB], F32)
            nc.vector.tensor_tensor(sp[:], gr[:], ga[:], OP.add)
            nc.vector.tensor_tensor(g1[:], sp[:], t_nega[:], OP.mult)
            eg = work.tile([D, NHB], F32)
            nc.scalar.activation(eg[:], g1[:], AF.Exp)

            kg = work.tile([D, NHB], F32)
            nc.vector.tensor_tensor(kg[:], kh[:], eg[:], OP.mult)
            qg = work.tile([D, NHB], F32)
            nc.vector.tensor_tensor(qg[:], qh[:], eg[:], OP.mult)

            # ---- qk = q_hat . k_hat per (b,h), broadcast along partitions -
            nc.vector.tensor_tensor(sq[:, 0:NHB], qh[:], kh[:], OP.mult)
            qkrow = psr.tile([1, NHB], F32)
            nc.tensor.matmul(qkrow[:], ones_c[:], sq[:, 0:NHB],
                             start=True, stop=True)
            qkrs = work.tile([1, NHB], F32)
            nc.vector.tensor_copy(qkrs[:], qkrow[:])
            qkb_ps = psb.tile([D, NHB], F32)
            nc.tensor.matmul(qkb_ps[:], ones_r[:], qkrs[:], start=True, stop=True)
            qkb = work.tile([D, NHB], F32)
            nc.vector.tensor_copy(qkb[:], qkb_ps[:])

            # sigmoid(beta) = 1/(1+exp(-beta)) — reuses the exp table
            bsig = work.tile([D, NHB], F32)
            nc.scalar.activation(bsig[:], t_beta[:], AF.Exp, scale=-1.0)
            nc.vector.tensor_scalar_add(bsig[:], bsig[:], 1.0)
            nc.vector.reciprocal(bsig[:], bsig[:])

            # ---- fold the delta-rule correction into one query vector -----
            # o = o1 + qk*b*(v - kv) = (qg - qk*b*kg) @ S + (qk*b)*v
            cc = work.tile([D, NHB], F32)
            nc.vector.tensor_tensor(cc[:], qkb[:], bsig[:], OP.mult)
            cv = work.tile([D, NHB], F32)
            nc.vector.tensor_tensor(cv[:], cc[:], v, OP.mult)
            mg = work.tile([D, NHB], F32)
            nc.vector.tensor_tensor(mg[:], cc[:], kg[:], OP.mult)
            nc.vector.tensor_tensor(mg[:], qg[:], mg[:], OP.subtract)
            # split mg hi/lo into fp16 to match the fp16 S pair; moving
            # operand columns: mgh = [mg_hi | mg_lo], mgz = [mg_hi | 0]
            mgh = work.tile([D, 2 * NHB], F16)
            mgh_v = mgh.rearrange("p (n two) -> p n two", two=2)
            nc.vector.tensor_copy(mgh_v[:, :, 0], mg[:])
            mghi32 = work.tile([D, NHB], F32)
            nc.vector.tensor_copy(mghi32[:], mgh_v[:, :, 0])
            nc.vector.tensor_tensor(mgh_v[:, :, 1], mg[:], mghi32[:],
                                    OP.subtract)
            mgz = work.tile([D, 2 * NHB], F16)
            nc.vector.memset(mgz[:], 0.0)
            mgz_v = mgz.rearrange("p (n two) -> p n two", two=2)
            nc.vector.tensor_copy(mgz_v[:, :, 0], mgh_v[:, :, 0])

            # ---- main loop: stream S hi/lo, one fused mat-vec per (b,h) ---
            # two batches per DMA chunk (2 MB) for DMA efficiency
            sr_hi = ssm_hi[:].rearrange("(c b) h k v -> c k (b h) v", b=2)
            sr_lo = ssm_lo[:].rearrange("(c b) h k v -> c k (b h) v", b=2)
            o_t = work.tile([D, NHB], F32)
            T0 = pso.tile([D, 2 * HV * GW], F32)
            T1 = pso.tile([D, 2 * HV * GW], F32)
            Tg = (T0, T1)

            v_v = cv[:].rearrange("p (h b) -> p h b", b=BC)
            o_v = o_t[:].rearrange("p (h b) -> p h b", b=BC)

            for c in range(BC // 2):
                Sh = spool.tile([D, 2 * HV, D], F16, name="Sh", tag="Sh")
                nc.sync.dma_start(Sh[:], sr_hi[c])
                Sl = spool.tile([D, 2 * HV, D], F16, name="Sl", tag="Sl")
                nc.sync.dma_start(Sl[:], sr_lo[c])
                for bi in range(2):
                    b = 2 * c + bi
                    grp, bl = divmod(b, GW)
                    for h in range(HV):
                        col = 2 * (h * GW + bl)
                        bh = 2 * (h * BC + b)
                        hh = bi * HV + h
                        # col 2i   = mg_hi@S_hi + mg_hi@S_lo
                        # col 2i+1 = mg_lo@S_hi + 0
                        nc.tensor.matmul(
                            Tg[grp][:, col:col + 2], Sh[:, hh, :],
                            mgh[:, bh:bh + 2], start=True, stop=False)
                        nc.tensor.matmul(
                            Tg[grp][:, col:col + 2], Sl[:, hh, :],
                            mgz[:, bh:bh + 2], start=False, stop=True)
                    if bl == GW - 1:
                        Tv = Tg[grp].rearrange("p (h bl two) -> p h bl two",
                                               bl=GW, two=2)
                        bsel = slice(grp * GW, (grp + 1) * GW)
                        # o = (col0 + col1) + c*v ; one PSUM operand per op
                        ot = work.tile([D, HV, GW], F32, name="ot", tag="ot")
                        nc.vector.scalar_tensor_tensor(
                            ot[:], Tv[:, :, :, 0], 1.0, v_v[:, :, bsel],
                            OP.mult, OP.add)
                        nc.vector.tensor_tensor(o_v[:, :, bsel], ot[:],
                                                Tv[:, :, :, 1], OP.add)

            nc.sync.dma_start(o_out[:], o_t[:])

    nc.compile()
    return nc


def _prep_act(a):
    """[bc, sec*32*128] activation slice -> [128 d, sec*32*bc] layout A."""
    bc = a.shape[0]
    return np.ascontiguousarray(
        a.reshape(bc, SEC, HV, D).transpose(3, 1, 2, 0).reshape(D, SEC * HV * bc))


def _prep_inputs(mixed_qkv, forget_gate, beta, conv_state, conv_weights,
                 ssm_state, A_log, dt_bias):
    mixed_qkv = np.asarray(mixed_qkv, np.float32)
    forget_gate = np.asarray(forget_gate, np.float32)
    beta = np.asarray(beta, np.float32)
    conv_state = np.asarray(conv_state, np.float32)
    conv_weights = np.asarray(conv_weights, np.float32)
    ssm_state = np.asarray(ssm_state, np.float32)
    A_log = np.asarray(A_log, np.float32)
    dt_bias = np.asarray(dt_bias, np.float32)

    # shared (weight) tensors
    wr = conv_weights.reshape(SEC, HV, D, CK).transpose(3, 2, 0, 1)  # [4,d,sec,h]
    wr = np.broadcast_to(wr[..., None], (CK, D, SEC, HV, BC))
    wrep = np.ascontiguousarray(
        wr.transpose(1, 0, 2, 3, 4).reshape(D, CK * SEC * HV * BC))
    dtb = np.ascontiguousarray(
        np.broadcast_to(dt_bias.reshape(HV, D).T[:, :, None],
                        (D, HV, BC)).reshape(D, NHB))
    nega = np.ascontiguousarray(
        np.broadcast_to((-np.exp(A_log))[None, :, None],
                        (D, HV, BC)).reshape(D, NHB))

    in_maps = []
    for c in range(NCORES):
        cs = slice(c * BC, (c + 1) * BC)
        cst = conv_state[cs]  # [BC, QKV, 3]
        cstp = np.concatenate([_prep_act(cst[:, :, j]) for j in range(CK - 1)],
                              axis=1)
        fgp = np.ascontiguousarray(
            forget_gate[cs].reshape(BC, HV, D).transpose(2, 1, 0).reshape(D, NHB))
        betar = np.ascontiguousarray(
            np.broadcast_to(beta[cs].T[None, :, :], (D, HV, BC)).reshape(D, NHB))
        ssm_c = ssm_state[cs]
        ssm_hi = ssm_c.astype(np.float16)
        ssm_lo = (ssm_c - ssm_hi.astype(np.float32)).astype(np.float16)
        in_maps.append({
            "xq": _prep_act(mixed_qkv[cs]),
            "cst": np.ascontiguousarray(cstp),
            "wrep": wrep,
            "aux": np.ascontiguousarray(
                np.concatenate([fgp, dtb, nega, betar], axis=1)),
            "ssm_hi": np.ascontiguousarray(ssm_hi),
            "ssm_lo": np.ascontiguousarray(ssm_lo),
        })
    return in_maps


def run(trace=False, **inputs):
    if "nc" not in _CACHE:
        _CACHE["nc"] = _build_nc()
    nc = _CACHE["nc"]
    in_maps = _prep_inputs(**inputs)
    res = run_bass_kernel_spmd(nc, in_maps, list(range(NCORES)), trace=trace)
    outs = []
    for c in range(NCORES):
        oc = np.asarray(res.results[c]["o_out"])  # [128, 512]
        outs.append(oc.reshape(D, HV, BC).transpose(2, 1, 0))  # [BC, HV, D]
    return np.concatenate(outs, axis=0), res


def kernel(**inputs) -> np.ndarray:
    out, _ = run(trace=False, **inputs)
    return out



# revision 7
# speedup vs baseline: 1.4887x; 1.4887x over previous
"""KimiLinear KDA decode step — Trainium2 Bass kernel (8 NeuronCores).

Problem: B=128 decode batch, HK=HV=32 heads, D=128 head dim, K=4 causal conv.
  1. per-channel causal conv1d update + silu over mixed_qkv (12288 channels)
  2. split q/k/v, l2norm(q)*D^-0.5, l2norm(k)
  3. fused KDA gate g = -exp(A_log)*softplus(forget_gate + dt_bias), b=sigmoid(beta)
  4. gated delta-rule readout:
       S' = S * exp(g);  kv = k @ S';  delta = (v - kv)*b
       o  = q @ (S' + k (x) delta) = (qg - qk*b*kg) @ S + (qk*b)*v
     with qg = q_hat*eg, kg = k_hat*eg folded into ONE query vector mg.

Sharding: data-parallel over batch — 16 batches per core; each core handles all
32 heads of its batch slice with zero cross-core communication.

Layout (the key change vs a matmul formulation): everything is (b,h)-major.
A "chunk" is 4 batches x 32 heads = 128 (b,h) pairs mapped to the 128 SBUF
partitions.  The SSM state is staged host-side as [chunk, bh, v, k] fp16 so
the per-(b,h) readout o[v] = sum_k mg[k] * S[v,k] becomes
  - one Vector-engine tensor_tensor multiply P = S * broadcast(mg)   (2x mode)
  - a pairwise in-place add-tree over k (6 fp16 levels + 1 fp32 level)
No TensorEngine matmuls at all: the per-(b,h) stationary loads + array-drain
latency (~175ns each, 512-1024 of them) were the previous bottleneck.  The
preamble (conv/norms/gates) is also bh-major, so all reductions are free-axis
tensor_reduce and all per-(b,h) scalars are native [P,1] broadcasts.

The SSM state is quantized host-side to a SINGLE fp16 copy (not a hi/lo pair):
~5e-4 relative error, halving the dominant HBM stream (33.5 -> 16.8 MB/core).
Per-core HBM traffic ~19.3 MB => ~52 us at ~370 GB/s; the DVE work (~35 us)
hides under the DMA stream.
"""

import numpy as np

import concourse.bass as bass
import concourse.bacc as bacc
import concourse.mybir as mybir
from concourse.tile import TileContext
from concourse.bass_utils import run_bass_kernel_spmd

F32 = mybir.dt.float32
F16 = mybir.dt.float16
AF = mybir.ActivationFunctionType
OP = mybir.AluOpType

NCORES = 8
B, HK, HV, D, CK = 128, 32, 32, 128, 4
SEC = 3                      # q | k | v sections
BC = B // NCORES             # batches per core = 16
NC_CH = 4                    # chunks per core (4 batches x 32 heads = 128 bh)
VS = 2                       # v-split per chunk (sub-chunk = 64 v rows)
SUBC = NC_CH * VS            # 8 sub-chunks
VH = D // VS                 # 64 v rows per sub-chunk
QKV = (2 * HK + HV) * D      # 12288

_CACHE = {}


def _build_nc():
    nc = bacc.Bacc("TRN2", target_bir_lowering=False, debug=False)
    # S stream: [sub-chunk, bh, v, k] fp16, fully contiguous per sub-chunk
    ssm_t = nc.declare_dram_parameter("ssm_t", [SUBC, D, VH, D], F16, isOutput=False)
    # conv window taps 0..2: [bh, (tap, c, sec, d)] fp16
    cst = nc.declare_dram_parameter("cst", [D, (CK - 1) * NC_CH * SEC * D], F16,
                                    isOutput=False)
    # current input: [bh, (c, sec, d)] fp16
    xq = nc.declare_dram_parameter("xq", [D, NC_CH * SEC * D], F16, isOutput=False)
    # conv weights: [bh(h replicated), (tap, sec, d)] fp16
    cw = nc.declare_dram_parameter("cw", [D, CK * SEC * D], F16, isOutput=False)
    # forget gate: [bh, (c, d)] fp32
    fgx = nc.declare_dram_parameter("fgx", [D, NC_CH * D], F32, isOutput=False)
    # dt_bias: [bh(h replicated), d] fp32
    dtb = nc.declare_dram_parameter("dtb", [D, D], F32, isOutput=False)
    # misc per-(b,h) scalars: col0 = -exp(A_log), col1..4 = beta per chunk
    misc = nc.declare_dram_parameter("misc", [D, 8], F32, isOutput=False)
    # output: [sub-chunk, bh, v-half] fp32
    o_out = nc.declare_dram_parameter("o_out", [SUBC, D, VH], F32, isOutput=True)

    CSD = NC_CH * SEC * D    # 1536
    NHB = NC_CH * D          # 512 free cols for (c, d)

    with TileContext(nc) as tc:
        with (
            tc.tile_pool(name="const", bufs=1) as const,
            tc.tile_pool(name="work", bufs=1) as work,
            tc.tile_pool(name="spool", bufs=4) as spool,
        ):
            # ---- input staging (scalar/ACT HWDGE ring; S stream on sync) ---
            t_cst = const.tile([D, (CK - 1) * CSD], F16)
            nc.scalar.dma_start(t_cst[:], cst[:])
            t_xq = const.tile([D, CSD], F16)
            nc.scalar.dma_start(t_xq[:], xq[:])
            t_cw = const.tile([D, CK * SEC * D], F16)
            nc.scalar.dma_start(t_cw[:], cw[:])
            cw_v = t_cw[:].rearrange("p (t s d) -> p t s d", t=CK, s=SEC)
            t_fg = const.tile([D, NC_CH * D], F32)
            nc.scalar.dma_start(t_fg[:], fgx[:])
            fg_v = t_fg[:].rearrange("p (c d) -> p c d", c=NC_CH)
            t_dtb = const.tile([D, D], F32)
            nc.scalar.dma_start(t_dtb[:], dtb[:])
            t_misc = const.tile([D, 8], F32)
            nc.scalar.dma_start(t_misc[:], misc[:])

            # ---- ACT table warm-ups (hidden under the input DMAs) ----------
            warm = work.tile([1, 4], F32)
            nc.vector.memset(warm[:], 1.0)
            for af in (AF.Silu, AF.Sqrt, AF.Ln, AF.Exp):
                nc.scalar.activation(warm[:, 0:1], warm[:, 1:2], af)

            # ---- causal conv1d single-step + silu (fp16, 2x DVE mode) ------
            cst_v = t_cst[:].rearrange("p (t f) -> p t f", t=CK - 1)
            acc = work.tile([D, CSD], F16)
            tmp = work.tile([D, CSD], F16)

            def wb(j):
                # weights for tap j broadcast over the chunk dim c
                return cw_v[:, j, None, :, :].to_broadcast((D, NC_CH, SEC, D))

            cst3 = cst_v.rearrange("p t (c s d) -> p t c s d", c=NC_CH, s=SEC)
            acc_v = acc[:].rearrange("p (c s d) -> p c s d", c=NC_CH, s=SEC)
            tmp_v = tmp[:].rearrange("p (c s d) -> p c s d", c=NC_CH, s=SEC)
            xq_v = t_xq[:].rearrange("p (c s d) -> p c s d", c=NC_CH, s=SEC)
            nc.vector.tensor_tensor(acc_v[:], cst3[:, 0], wb(0), OP.mult)
            for j in (1, 2):
                nc.vector.tensor_tensor(tmp_v[:], cst3[:, j], wb(j), OP.mult)
                nc.vector.tensor_tensor(acc[:], acc[:], tmp[:], OP.add)
            nc.vector.tensor_tensor(tmp_v[:], xq_v[:], wb(3), OP.mult)
            nc.vector.tensor_tensor(acc[:], acc[:], tmp[:], OP.add)

            x = work.tile([D, NC_CH, SEC, D], F16)
            nc.scalar.activation(x[:], acc_v[:], AF.Silu)
            xqs = x[:, :, 0, :]       # [p, c, d] strided views
            xks = x[:, :, 1, :]
            xvs = x[:, :, 2, :]

            # ---- l2 norms (free-axis reduce; scalars are per-partition) ----
            sq = work.tile([D, NC_CH, D], F16)
            nq = work.tile([D, NC_CH], F32)
            nk = work.tile([D, NC_CH], F32)
            nc.vector.tensor_tensor(sq[:], xqs, xqs, OP.mult)
            nc.vector.tensor_reduce(nq[:], sq[:], mybir.AxisListType.X, OP.add)
            nc.vector.tensor_tensor(sq[:], xks, xks, OP.mult)
            nc.vector.tensor_reduce(nk[:], sq[:], mybir.AxisListType.X, OP.add)
            nc.vector.tensor_scalar_add(nq[:], nq[:], 1e-6)
            nc.vector.tensor_scalar_add(nk[:], nk[:], 1e-6)
            rq = work.tile([D, NC_CH], F32)
            rk = work.tile([D, NC_CH], F32)
            nc.vector.reciprocal(rq[:], nq[:])
            nc.vector.reciprocal(rk[:], nk[:])
            # rsqrt = sqrt(1/x); q also gets the D^-0.5 scale folded in
            nc.scalar.activation(rq[:], rq[:], AF.Sqrt, scale=1.0 / D)
            nc.scalar.activation(rk[:], rk[:], AF.Sqrt)

            # ---- KDA gate: eg = exp(-exp(A_log) * softplus(fg + dt_bias)) --
            # softplus(x) = relu(x) + ln(1 + exp(-|x|)) — no softplus table on
            # this compiler; ln/exp share one table with the final exp.
            g1 = work.tile([D, NC_CH, D], F32)
            nc.vector.tensor_tensor(
                g1[:], fg_v[:], t_dtb[:, None, :].to_broadcast((D, NC_CH, D)),
                OP.add)
            ga = work.tile([D, NC_CH, D], F32)
            nc.scalar.activation(ga[:], g1[:], AF.Abs)
            nc.scalar.activation(ga[:], ga[:], AF.Exp, scale=-1.0)
            nc.scalar.activation(ga[:], ga[:], AF.Ln, bias=1.0)
            nc.vector.tensor_scalar_max(g1[:], g1[:], 0.0)
            nc.vector.tensor_tensor(g1[:], g1[:], ga[:], OP.add)
            nc.vector.tensor_scalar(g1[:], g1[:], t_misc[:, 0:1], None, OP.mult)
            eg = work.tile([D, NC_CH, D], F16)
            nc.scalar.activation(eg[:], g1[:], AF.Exp)

            # ---- qk dot, beta sigmoid, fold scalars ------------------------
            qkr = work.tile([D, NC_CH], F32)
            nc.vector.tensor_tensor(sq[:], xqs, xks, OP.mult)
            nc.vector.tensor_reduce(qkr[:], sq[:], mybir.AxisListType.X, OP.add)
            # qk = qkraw * rq * rk  (scales factor out of the dot)
            nc.vector.tensor_tensor(qkr[:], qkr[:], rq[:], OP.mult)
            nc.vector.tensor_tensor(qkr[:], qkr[:], rk[:], OP.mult)
            # b = sigmoid(beta) = 1/(1+exp(-beta))
            bsig = work.tile([D, NC_CH], F32)
            nc.scalar.activation(bsig[:], t_misc[:, 1:1 + NC_CH], AF.Exp,
                                 scale=-1.0)
            nc.vector.tensor_scalar_add(bsig[:], bsig[:], 1.0)
            nc.vector.reciprocal(bsig[:], bsig[:])
            cvb = work.tile([D, NC_CH], F32)      # qk*b      (for the +v term)
            nc.vector.tensor_tensor(cvb[:], qkr[:], bsig[:], OP.mult)
            mgs = work.tile([D, NC_CH], F32)      # -qk*b*rk  (fold into k)
            nc.vector.tensor_tensor(mgs[:], cvb[:], rk[:], OP.mult)
            nc.vector.tensor_scalar(mgs[:], mgs[:], -1.0, None, OP.mult)

            # ---- mg = (q*rq - qk*b*rk*k) * eg  -----------------------------
            qh = work.tile([D, NC_CH, D], F16)
            mg = work.tile([D, NC_CH, D], F16)
            for c in range(NC_CH):
                nc.vector.tensor_scalar(qh[:, c, :], xqs[:, c, :],
                                        rq[:, c:c + 1], None, OP.mult)
                nc.vector.scalar_tensor_tensor(
                    mg[:, c, :], xks[:, c, :], mgs[:, c:c + 1], qh[:, c, :],
                    OP.mult, OP.add)
            nc.vector.tensor_tensor(mg[:], mg[:], eg[:], OP.mult)

            # ---- main loop: stream S, batched gemv on the Vector engine ----
            P = work.tile([D, VH, D], F16)
            o_t = work.tile([D, NC_CH, D], F32)
            o_v = o_t[:].rearrange("p c (vs vh) -> p c vs vh", vs=VS)

            for s in range(SUBC):
                c, vh = divmod(s, VS)
                St = spool.tile([D, VH, D], F16, name="St", tag="St")
                nc.sync.dma_start(St[:], ssm_t[s])
                # P[p, v, k] = S[p, v, k] * mg[p, k]
                nc.vector.tensor_tensor(
                    P[:], St[:],
                    mg[:, c, None, :].to_broadcast((D, VH, D)), OP.mult)
                # pairwise in-place add-tree over k: 128 -> 2 in fp16
                w = D // 2
                while w >= 2:
                    nc.vector.tensor_tensor(
                        P[:, :, 0:w], P[:, :, 0:w], P[:, :, w:2 * w], OP.add)
                    w //= 2
                osl = o_v[:, c, vh, :]
                nc.vector.tensor_tensor(osl, P[:, :, 0], P[:, :, 1], OP.add)
                # o += (qk*b) * v
                nc.vector.scalar_tensor_tensor(
                    osl, xvs[:, c, vh * VH:(vh + 1) * VH], cvb[:, c:c + 1],
                    osl, OP.mult, OP.add)
                nc.scalar.dma_start(o_out[s], osl)

    nc.compile()
    return nc


def _prep_inputs(mixed_qkv, forget_gate, beta, conv_state, conv_weights,
                 ssm_state, A_log, dt_bias):
    mixed_qkv = np.asarray(mixed_qkv, np.float32)
    forget_gate = np.asarray(forget_gate, np.float32)
    beta = np.asarray(beta, np.float32)
    conv_state = np.asarray(conv_state, np.float32)
    conv_weights = np.asarray(conv_weights, np.float32)
    ssm_state = np.asarray(ssm_state, np.float32)
    A_log = np.asarray(A_log, np.float32)
    dt_bias = np.asarray(dt_bias, np.float32)

    # shared (weight-like) tensors
    # cw: [12288, 4] -> [h, tap, sec, d] replicated over b4 -> [128, 1536]
    w = conv_weights.reshape(SEC, HV, D, CK).transpose(1, 3, 0, 2)
    cw = np.broadcast_to(w[None], (4, HV, CK, SEC, D)).reshape(D, CK * SEC * D)
    cw = np.ascontiguousarray(cw, dtype=np.float16)
    # dtb: [4096] -> [h, d] replicated over b4 -> [128, 128]
    dtbp = np.ascontiguousarray(
        np.broadcast_to(dt_bias.reshape(HV, D)[None], (4, HV, D)).reshape(D, D))
    nega = np.broadcast_to((-np.exp(A_log))[None], (4, HV)).reshape(D)

    in_maps = []
    for ci in range(NCORES):
        cs = slice(ci * BC, (ci + 1) * BC)
        # S: [16, 32, 128k, 128v] -> [c, b4, h, v, k] -> [s=(c,vs), p, vh, k]
        s = ssm_state[cs].reshape(NC_CH, 4, HV, D, D)
        s = s.transpose(0, 1, 2, 4, 3).reshape(NC_CH, D, VS, VH, D)
        s = s.transpose(0, 2, 1, 3, 4).reshape(SUBC, D, VH, D)
        s = np.ascontiguousarray(s).astype(np.float16)
        # conv state: [16, 12288, 3] -> [p, (tap, c, sec, d)]
        cstp = conv_state[cs].reshape(NC_CH, 4, SEC, HV, D, CK - 1)
        cstp = cstp.transpose(1, 3, 5, 0, 2, 4).reshape(D, (CK - 1) * NC_CH * SEC * D)
        cstp = np.ascontiguousarray(cstp).astype(np.float16)
        # current input: [16, 12288] -> [p, (c, sec, d)]
        xqp = mixed_qkv[cs].reshape(NC_CH, 4, SEC, HV, D)
        xqp = xqp.transpose(1, 3, 0, 2, 4).reshape(D, NC_CH * SEC * D)
        xqp = np.ascontiguousarray(xqp).astype(np.float16)
        # forget gate: [16, 4096] -> [p, (c, d)] fp32
        fgp = forget_gate[cs].reshape(NC_CH, 4, HV, D)
        fgp = np.ascontiguousarray(
            fgp.transpose(1, 2, 0, 3).reshape(D, NC_CH * D))
        # misc: [p, 8] = [nega | beta(c=0..3) | pad]
        mi = np.zeros((D, 8), np.float32)
        mi[:, 0] = nega
        bet = beta[cs].reshape(NC_CH, 4, HV).transpose(1, 2, 0).reshape(D, NC_CH)
        mi[:, 1:1 + NC_CH] = bet
        in_maps.append({
            "ssm_t": s,
            "cst": cstp,
            "xq": xqp,
            "cw": cw,
            "fgx": fgp,
            "dtb": dtbp,
            "misc": mi,
        })
    return in_maps


def run(trace=False, **inputs):
    if "nc" not in _CACHE:
        _CACHE["nc"] = _build_nc()
    nc = _CACHE["nc"]
    in_maps = _prep_inputs(**inputs)
    res = run_bass_kernel_spmd(nc, in_maps, list(range(NCORES)), trace=trace)
    outs = []
    for ci in range(NCORES):
        oc = np.asarray(res.results[ci]["o_out"])  # [8, 128, 64]
        oc = oc.reshape(NC_CH, VS, 4, HV, VH).transpose(0, 2, 3, 1, 4)
        outs.append(oc.reshape(BC, HV, D))
    return np.concatenate(outs, axis=0), res


def kernel(**inputs) -> np.ndarray:
    out, _ = run(trace=False, **inputs)
    return out


# revision 10
# speedup vs baseline: 1.8816x; 1.2639x over previous
"""KimiLinear KDA decode step — Trainium2 Bass kernel (8 NeuronCores).

Problem: B=128 decode batch, HK=HV=32 heads, D=128 head dim, K=4 causal conv.
  1. per-channel causal conv1d update + silu over mixed_qkv (12288 channels)
  2. split q/k/v, l2norm(q)*D^-0.5, l2norm(k)
  3. fused KDA gate g = -exp(A_log)*softplus(forget_gate + dt_bias), b=sigmoid(beta)
  4. gated delta-rule readout folded into ONE query vector:
       o = (q_hat*eg - qk*b*(k_hat*eg)) @ S + (qk*b)*v    (eg = exp(g))

Sharding: data-parallel over batch — 16 batches per core, zero cross-core
communication.  Within a core the 16 batches form 4 "chunks" of 4 batches x
32 heads = 128 (b,h) pairs.

The readout o[v] = sum_k mg[k]*S[k,v] for 512 independent (b,h) pairs is
split across BOTH compute engines so it hides under the fp16 S stream
(~19 MB/core total HBM traffic):
  - PE path (chunks 0-1): S staged [k, bh, v]; one 128x128 fp16 stationary
    matmul per (b,h) with the folded query as the single moving column.
    Per-MM cost is the array drain latency (~170ns); 256 MMs ~= 44us.
  - DVE path (chunks 2-3): S staged [bh(partitions), v, k]; one
    tensor_tensor multiply against the broadcast mg row + a pairwise
    in-place add-tree over k.  (Measured: DVE runs 1x only — no 16-bit
    packing on this HW — so ~0.54ns/free-elem; ~10us per 2MB sub-chunk.)
The preamble (conv/norms/gates) is bh-major, so reductions are free-axis
tensor_reduce and per-(b,h) scalars are native [P,1] broadcasts; the PE path
gets its k-major query/correction vectors via two 128x128 PE transposes per
chunk.  S is quantized host-side to a single fp16 copy (~5e-4 rel err,
halving the dominant stream vs fp32).
"""

import numpy as np

import concourse.bass as bass
import concourse.bacc as bacc
import concourse.mybir as mybir
from concourse.tile import TileContext
from concourse.bass_utils import run_bass_kernel_spmd

F32 = mybir.dt.float32
F16 = mybir.dt.float16
AF = mybir.ActivationFunctionType
OP = mybir.AluOpType

NCORES = 8
B, HK, HV, D, CK = 128, 32, 32, 128, 4
SEC = 3                      # q | k | v sections
BC = B // NCORES             # batches per core = 16
NC_CH = 4                    # chunks per core (4 batches x 32 heads = 128 bh)
PE_CH = 2                    # chunks handled by the tensor engine
VS = 2                       # v-split per DVE chunk
VH = D // VS                 # 64 v rows per DVE sub-chunk
NSUB = (NC_CH - PE_CH) * VS  # 4 DVE sub-chunks
NHALF = PE_CH * 2            # 4 PE half-chunks (64 bh columns each)
QKV = (2 * HK + HV) * D      # 12288

_CACHE = {}


def _build_nc():
    nc = bacc.Bacc("TRN2", target_bir_lowering=False, debug=False)
    # DVE S stream: [sub-chunk, bh, v-half, k] fp16, contiguous per sub-chunk
    s_dve = nc.declare_dram_parameter("s_dve", [NSUB, D, VH, D], F16, isOutput=False)
    # PE S stream: [half-chunk, k, bh(64), v] fp16, contiguous per half-chunk
    s_pe = nc.declare_dram_parameter("s_pe", [NHALF, D, 64 * D], F16, isOutput=False)
    cst = nc.declare_dram_parameter("cst", [D, (CK - 1) * NC_CH * SEC * D], F16,
                                    isOutput=False)
    xq = nc.declare_dram_parameter("xq", [D, NC_CH * SEC * D], F16, isOutput=False)
    cw = nc.declare_dram_parameter("cw", [D, CK * SEC * D], F16, isOutput=False)
    fgx = nc.declare_dram_parameter("fgx", [D, NC_CH * D], F32, isOutput=False)
    dtb = nc.declare_dram_parameter("dtb", [D, D], F32, isOutput=False)
    misc = nc.declare_dram_parameter("misc", [D, 8], F32, isOutput=False)
    ident = nc.declare_dram_parameter("ident", [D, D], F16, isOutput=False)
    # outputs: DVE part [sub, bh, v-half]; PE part [half, v, bh-col]
    o_dve = nc.declare_dram_parameter("o_dve", [NSUB, D, VH], F32, isOutput=True)
    o_pe = nc.declare_dram_parameter("o_pe", [NHALF, D, 64], F32, isOutput=True)

    CSD = NC_CH * SEC * D    # 1536

    with TileContext(nc) as tc:
        with (
            tc.tile_pool(name="const", bufs=1) as const,
            tc.tile_pool(name="work", bufs=1) as work,
            tc.tile_pool(name="sdve", bufs=2) as sdve,
            tc.tile_pool(name="spe", bufs=2) as spe,
            tc.tile_pool(name="pst", bufs=2, space="PSUM") as pst,
            tc.tile_pool(name="psm", bufs=2, space="PSUM") as psm,
        ):
            # ---- input staging (scalar ring; S stream interleaved on sync) -
            t_cst = const.tile([D, (CK - 1) * CSD], F16)
            nc.scalar.dma_start(t_cst[:], cst[:])
            t_xq = const.tile([D, CSD], F16)
            nc.scalar.dma_start(t_xq[:], xq[:])
            t_cw = const.tile([D, CK * SEC * D], F16)
            nc.scalar.dma_start(t_cw[:], cw[:])
            cw_v = t_cw[:].rearrange("p (t s d) -> p t s d", t=CK, s=SEC)
            t_fg = const.tile([D, NC_CH * D], F32)
            nc.scalar.dma_start(t_fg[:], fgx[:])
            fg_v = t_fg[:].rearrange("p (c d) -> p c d", c=NC_CH)
            t_dtb = const.tile([D, D], F32)
            nc.scalar.dma_start(t_dtb[:], dtb[:])
            t_misc = const.tile([D, 8], F32)
            nc.scalar.dma_start(t_misc[:], misc[:])
            t_id = const.tile([D, D], F16)
            nc.scalar.dma_start(t_id[:], ident[:])

            # S stream: alternate DVE sub-chunks and PE half-chunks (2.1 MB
            # each) so both engines consume the stream concurrently.
            s_tiles = []
            for i in range(NSUB):
                Sd = sdve.tile([D, VH, D], F16, name=f"Sd{i}", tag="Sd")
                nc.sync.dma_start(Sd[:], s_dve[i])
                Sp = spe.tile([D, 64, D], F16, name=f"Sp{i}", tag="Sp")
                nc.sync.dma_start(Sp[:], s_pe[i])
                s_tiles.append((Sd, Sp))

            # ---- causal conv1d single-step + silu -------------------------
            cst_v = t_cst[:].rearrange("p (t f) -> p t f", t=CK - 1)
            cst3 = cst_v.rearrange("p t (c s d) -> p t c s d", c=NC_CH, s=SEC)
            acc = work.tile([D, CSD], F16)
            tmp = work.tile([D, CSD], F16)
            acc_v = acc[:].rearrange("p (c s d) -> p c s d", c=NC_CH, s=SEC)
            tmp_v = tmp[:].rearrange("p (c s d) -> p c s d", c=NC_CH, s=SEC)
            xq_v = t_xq[:].rearrange("p (c s d) -> p c s d", c=NC_CH, s=SEC)

            def wb(j):
                return cw_v[:, j, None, :, :].to_broadcast((D, NC_CH, SEC, D))

            nc.vector.tensor_tensor(acc_v[:], cst3[:, 0], wb(0), OP.mult)
            for j in (1, 2):
                nc.vector.tensor_tensor(tmp_v[:], cst3[:, j], wb(j), OP.mult)
                nc.vector.tensor_tensor(acc[:], acc[:], tmp[:], OP.add)
            nc.vector.tensor_tensor(tmp_v[:], xq_v[:], wb(3), OP.mult)
            nc.vector.tensor_tensor(acc[:], acc[:], tmp[:], OP.add)

            x = work.tile([D, NC_CH, SEC, D], F16)
            nc.scalar.activation(x[:], acc_v[:], AF.Silu)
            xqs = x[:, :, 0, :]
            xks = x[:, :, 1, :]
            xvs = x[:, :, 2, :]

            # gate input (independent of conv; Abs groups with the silu table)
            g1 = work.tile([D, NC_CH, D], F32)
            nc.vector.tensor_tensor(
                g1[:], fg_v[:], t_dtb[:, None, :].to_broadcast((D, NC_CH, D)),
                OP.add)
            ga = work.tile([D, NC_CH, D], F32)
            nc.scalar.activation(ga[:], g1[:], AF.Abs)

            # ---- l2 norms --------------------------------------------------
            sq = work.tile([D, NC_CH, D], F16)
            nq = work.tile([D, NC_CH], F32)
            nk = work.tile([D, NC_CH], F32)
            nc.vector.tensor_tensor(sq[:], xqs, xqs, OP.mult)
            nc.vector.tensor_reduce(nq[:], sq[:], mybir.AxisListType.X, OP.add)
            nc.vector.tensor_tensor(sq[:], xks, xks, OP.mult)
            nc.vector.tensor_reduce(nk[:], sq[:], mybir.AxisListType.X, OP.add)
            nc.vector.tensor_scalar_add(nq[:], nq[:], 1e-6)
            nc.vector.tensor_scalar_add(nk[:], nk[:], 1e-6)
            rq = work.tile([D, NC_CH], F32)
            rk = work.tile([D, NC_CH], F32)
            nc.vector.reciprocal(rq[:], nq[:])
            nc.vector.reciprocal(rk[:], nk[:])
            # rsqrt = sqrt(1/x); q also gets the D^-0.5 scale folded in
            nc.scalar.activation(rq[:], rq[:], AF.Sqrt, scale=1.0 / D)
            nc.scalar.activation(rk[:], rk[:], AF.Sqrt)

            # ---- KDA gate: softplus(x) = relu(x) + ln(1+exp(-|x|)) --------
            nc.scalar.activation(ga[:], ga[:], AF.Exp, scale=-1.0)
            nc.scalar.activation(ga[:], ga[:], AF.Ln, bias=1.0)
            nc.vector.tensor_scalar_max(g1[:], g1[:], 0.0)
            nc.vector.tensor_tensor(g1[:], g1[:], ga[:], OP.add)
            nc.vector.tensor_scalar(g1[:], g1[:], t_misc[:, 0:1], None, OP.mult)
            eg = work.tile([D, NC_CH, D], F16)
            nc.scalar.activation(eg[:], g1[:], AF.Exp)
            # b = sigmoid(beta) = 1/(1+exp(-beta))
            bsig = work.tile([D, NC_CH], F32)
            nc.scalar.activation(bsig[:], t_misc[:, 1:1 + NC_CH], AF.Exp,
                                 scale=-1.0)
            nc.vector.tensor_scalar_add(bsig[:], bsig[:], 1.0)
            nc.vector.reciprocal(bsig[:], bsig[:])

            # ---- fold per-(b,h) scalars -----------------------------------
            qkr = work.tile([D, NC_CH], F32)
            nc.vector.tensor_tensor(sq[:], xqs, xks, OP.mult)
            nc.vector.tensor_reduce(qkr[:], sq[:], mybir.AxisListType.X, OP.add)
            nc.vector.tensor_tensor(qkr[:], qkr[:], rq[:], OP.mult)
            nc.vector.tensor_tensor(qkr[:], qkr[:], rk[:], OP.mult)
            cvb = work.tile([D, NC_CH], F32)      # qk*b      (for the +v term)
            nc.vector.tensor_tensor(cvb[:], qkr[:], bsig[:], OP.mult)
            mgs = work.tile([D, NC_CH], F32)      # -qk*b*rk  (fold into k)
            nc.vector.tensor_tensor(mgs[:], cvb[:], rk[:], OP.mult)
            nc.vector.tensor_scalar(mgs[:], mgs[:], -1.0, None, OP.mult)

            # ---- mg = (q*rq - qk*b*rk*k) * eg  -----------------------------
            qh = work.tile([D, NC_CH, D], F16)
            mg = work.tile([D, NC_CH, D], F16)
            for c in range(NC_CH):
                nc.vector.tensor_scalar(qh[:, c, :], xqs[:, c, :],
                                        rq[:, c:c + 1], None, OP.mult)
                nc.vector.scalar_tensor_tensor(
                    mg[:, c, :], xks[:, c, :], mgs[:, c:c + 1], qh[:, c, :],
                    OP.mult, OP.add)
            nc.vector.tensor_tensor(mg[:], mg[:], eg[:], OP.mult)

            # ---- PE-chunk prep: transpose mg and cvb*v to k/v-major --------
            mgT = []
            cvvT = []
            for c in range(PE_CH):
                tp = pst.tile([D, D], F16, name=f"tp{c}", tag="tp")
                nc.tensor.transpose(tp[:], mg[:, c, :], t_id[:])
                m16 = work.tile([D, D], F16, name=f"mgT{c}", tag="mgT")
                nc.vector.tensor_copy(m16[:], tp[:])
                mgT.append(m16)
                cvv = work.tile([D, D], F16, name=f"cvv{c}", tag="cvv")
                nc.vector.tensor_scalar(cvv[:], xvs[:, c, :], cvb[:, c:c + 1],
                                        None, OP.mult)
                tp2 = pst.tile([D, D], F16, name=f"tq{c}", tag="tp")
                nc.tensor.transpose(tp2[:], cvv[:], t_id[:])
                c32 = work.tile([D, D], F32, name=f"cvvT{c}", tag="cvvT")
                nc.vector.tensor_copy(c32[:], tp2[:])
                cvvT.append(c32)

            # ---- main loop: both engines stream their S halves -------------
            P = work.tile([D, VH, D], F16)
            for i in range(NSUB):
                Sd, Sp = s_tiles[i]
                # --- PE half-chunk i: 64 per-(b,h) stationary matmuls ------
                c, hf = divmod(i, 2)
                pso = psm.tile([D, 64], F32, name=f"pso{i}", tag="pso")
                for j in range(64):
                    col = hf * 64 + j
                    nc.tensor.matmul(pso[:, j:j + 1], Sp[:, j, :],
                                     mgT[c][:, col:col + 1],
                                     start=True, stop=True)
                ope = work.tile([D, 64], F32, name=f"ope{i}", tag="ope")
                nc.vector.tensor_tensor(
                    ope[:], pso[:], cvvT[c][:, hf * 64:hf * 64 + 64], OP.add)
                nc.scalar.dma_start(o_pe[i], ope[:])

                # --- DVE sub-chunk i: broadcast multiply + add-tree --------
                cd, vh = divmod(i, VS)
                cd += PE_CH
                nc.vector.tensor_tensor(
                    P[:], Sd[:],
                    mg[:, cd, None, :].to_broadcast((D, VH, D)), OP.mult)
                w = D // 2
                while w >= 2:
                    nc.vector.tensor_tensor(
                        P[:, :, 0:w], P[:, :, 0:w], P[:, :, w:2 * w], OP.add)
                    w //= 2
                od = work.tile([D, VH], F32, name=f"od{i}", tag="od")
                nc.vector.tensor_tensor(od[:], P[:, :, 0], P[:, :, 1], OP.add)
                nc.vector.scalar_tensor_tensor(
                    od[:], xvs[:, cd, vh * VH:(vh + 1) * VH], cvb[:, cd:cd + 1],
                    od[:], OP.mult, OP.add)
                nc.scalar.dma_start(o_dve[i], od[:])

    nc.compile()
    return nc


def _prep_inputs(mixed_qkv, forget_gate, beta, conv_state, conv_weights,
                 ssm_state, A_log, dt_bias):
    mixed_qkv = np.asarray(mixed_qkv, np.float32)
    forget_gate = np.asarray(forget_gate, np.float32)
    beta = np.asarray(beta, np.float32)
    conv_state = np.asarray(conv_state, np.float32)
    conv_weights = np.asarray(conv_weights, np.float32)
    ssm_state = np.asarray(ssm_state, np.float32)
    A_log = np.asarray(A_log, np.float32)
    dt_bias = np.asarray(dt_bias, np.float32)

    # shared (weight-like) tensors
    w = conv_weights.reshape(SEC, HV, D, CK).transpose(1, 3, 0, 2)
    cw = np.broadcast_to(w[None], (4, HV, CK, SEC, D)).reshape(D, CK * SEC * D)
    cw = np.ascontiguousarray(cw, dtype=np.float16)
    dtbp = np.ascontiguousarray(
        np.broadcast_to(dt_bias.reshape(HV, D)[None], (4, HV, D)).reshape(D, D))
    nega = np.broadcast_to((-np.exp(A_log))[None], (4, HV)).reshape(D)
    identity = np.eye(D, dtype=np.float16)

    in_maps = []
    for ci in range(NCORES):
        cs = slice(ci * BC, (ci + 1) * BC)
        ssm_c = ssm_state[cs]
        # PE chunks (batches 0..7): [half, k, (bh=64, v)]
        sp = ssm_c[0:8].reshape(PE_CH, 2, 2, HV, D, D)
        sp = sp.transpose(0, 1, 4, 2, 3, 5).reshape(NHALF, D, 64 * D)
        sp = np.ascontiguousarray(sp).astype(np.float16)
        # DVE chunks (batches 8..15): [sub, bh, v-half, k]
        sd = ssm_c[8:16].reshape(NC_CH - PE_CH, 4, HV, D, D)
        sd = sd.transpose(0, 1, 2, 4, 3).reshape(NC_CH - PE_CH, D, VS, VH, D)
        sd = sd.transpose(0, 2, 1, 3, 4).reshape(NSUB, D, VH, D)
        sd = np.ascontiguousarray(sd).astype(np.float16)
        # conv state: [16, 12288, 3] -> [p, (tap, c, sec, d)]
        cstp = conv_state[cs].reshape(NC_CH, 4, SEC, HV, D, CK - 1)
        cstp = cstp.transpose(1, 3, 5, 0, 2, 4).reshape(D, (CK - 1) * NC_CH * SEC * D)
        cstp = np.ascontiguousarray(cstp).astype(np.float16)
        xqp = mixed_qkv[cs].reshape(NC_CH, 4, SEC, HV, D)
        xqp = xqp.transpose(1, 3, 0, 2, 4).reshape(D, NC_CH * SEC * D)
        xqp = np.ascontiguousarray(xqp).astype(np.float16)
        fgp = forget_gate[cs].reshape(NC_CH, 4, HV, D)
        fgp = np.ascontiguousarray(
            fgp.transpose(1, 2, 0, 3).reshape(D, NC_CH * D))
        mi = np.zeros((D, 8), np.float32)
        mi[:, 0] = nega
        bet = beta[cs].reshape(NC_CH, 4, HV).transpose(1, 2, 0).reshape(D, NC_CH)
        mi[:, 1:1 + NC_CH] = bet
        in_maps.append({
            "s_dve": sd,
            "s_pe": sp,
            "cst": cstp,
            "xq": xqp,
            "cw": cw,
            "fgx": fgp,
            "dtb": dtbp,
            "misc": mi,
            "ident": identity,
        })
    return in_maps


def run(trace=False, **inputs):
    if "nc" not in _CACHE:
        _CACHE["nc"] = _build_nc()
    nc = _CACHE["nc"]
    in_maps = _prep_inputs(**inputs)
    res = run_bass_kernel_spmd(nc, in_maps, list(range(NCORES)), trace=trace)
    outs = []
    for ci in range(NCORES):
        r = res.results[ci]
        ope = np.asarray(r["o_pe"])   # [4, 128 v, 64 (b2,h)]
        ope = ope.reshape(PE_CH, 2, D, 2, HV).transpose(0, 1, 3, 4, 2)
        o_lo = ope.reshape(8, HV, D)
        odv = np.asarray(r["o_dve"])  # [4, 128 (b4,h), 64 vh]
        odv = odv.reshape(NC_CH - PE_CH, VS, 4, HV, VH).transpose(0, 2, 3, 1, 4)
        o_hi = odv.reshape(8, HV, D)
        outs.append(np.concatenate([o_lo, o_hi], axis=0))
    return np.concatenate(outs, axis=0), res


def kernel(**inputs) -> np.ndarray:
    out, _ = run(trace=False, **inputs)
    return out


# revision 11
# speedup vs baseline: 2.5197x; 1.3391x over previous
"""KimiLinear KDA decode step — Trainium2 Bass kernel (8 NeuronCores).

Problem: B=128 decode batch, HK=HV=32 heads, D=128 head dim, K=4 causal conv.
  1. per-channel causal conv1d update + silu over mixed_qkv (12288 channels)
  2. split q/k/v, l2norm(q)*D^-0.5, l2norm(k)
  3. fused KDA gate g = -exp(A_log)*softplus(forget_gate + dt_bias), b=sigmoid(beta)
  4. gated delta-rule readout folded into ONE query vector:
       o = (q_hat*eg - qk*b*(k_hat*eg)) @ S + (qk*b)*v    (eg = exp(g))

Sharding: data-parallel over batch — 16 batches per core, zero cross-core
communication.  Within a core the 16 batches form 4 "chunks" of 4 batches x
32 heads = 128 (b,h) pairs.

The readout o[v] = sum_k mg[k]*S[k,v] for 512 independent (b,h) pairs is
split across BOTH compute engines so it hides under the fp16 S stream
(~19 MB/core total HBM traffic):
  - PE path (chunks 0-1): S staged [k, bh, v]; one 128x128 fp16 stationary
    matmul per (b,h) with the folded query as the single moving column.
    Per-MM cost is the array drain latency (~170ns); 256 MMs ~= 44us.
  - DVE path (chunks 2-3): S staged [bh(partitions), v, k]; one
    tensor_tensor multiply against the broadcast mg row + a pairwise
    in-place add-tree over k.  (Measured: DVE runs 1x only — no 16-bit
    packing on this HW — so ~0.54ns/free-elem; ~10us per 2MB sub-chunk.)
The preamble (conv/norms/gates) is bh-major, so reductions are free-axis
tensor_reduce and per-(b,h) scalars are native [P,1] broadcasts; the PE path
gets its k-major query/correction vectors via two 128x128 PE transposes per
chunk.  S is quantized host-side to a single fp16 copy (~5e-4 rel err,
halving the dominant stream vs fp32).
"""

import numpy as np

import concourse.bass as bass
import concourse.bacc as bacc
import concourse.mybir as mybir
from concourse.tile import TileContext
from concourse.bass_utils import run_bass_kernel_spmd

F32 = mybir.dt.float32
F16 = mybir.dt.float16
AF = mybir.ActivationFunctionType
OP = mybir.AluOpType

NCORES = 8
B, HK, HV, D, CK = 128, 32, 32, 128, 4
SEC = 3                      # q | k | v sections
BC = B // NCORES             # batches per core = 16
NC_CH = 4                    # chunks per core (4 batches x 32 heads = 128 bh)
PE_CH = 2                    # chunks handled by the tensor engine
VS = 2                       # v-split per DVE chunk
VH = D // VS                 # 64 v rows per DVE sub-chunk
NSUB = (NC_CH - PE_CH) * VS  # 4 DVE sub-chunks
NHALF = PE_CH * 2            # 4 PE half-chunks (64 bh columns each)
QKV = (2 * HK + HV) * D      # 12288

_CACHE = {}


def _build_nc():
    nc = bacc.Bacc("TRN2", target_bir_lowering=False, debug=False)
    # DVE S stream: [sub-chunk, bh, v-half, k] fp16, contiguous per sub-chunk
    s_dve = nc.declare_dram_parameter("s_dve", [NSUB, D, VH, D], F16, isOutput=False)
    # PE S stream: [half-chunk, k, bh(64), v] fp16, contiguous per half-chunk
    s_pe = nc.declare_dram_parameter("s_pe", [NHALF, D, 64 * D], F16, isOutput=False)
    cst = nc.declare_dram_parameter("cst", [D, (CK - 1) * NC_CH * SEC * D], F16,
                                    isOutput=False)
    xq = nc.declare_dram_parameter("xq", [D, NC_CH * SEC * D], F16, isOutput=False)
    cw = nc.declare_dram_parameter("cw", [D, CK * SEC * D], F16, isOutput=False)
    fgx = nc.declare_dram_parameter("fgx", [D, NC_CH * D], F32, isOutput=False)
    dtb = nc.declare_dram_parameter("dtb", [D, D], F32, isOutput=False)
    misc = nc.declare_dram_parameter("misc", [D, 8], F32, isOutput=False)
    ident = nc.declare_dram_parameter("ident", [D, D], F16, isOutput=False)
    # outputs: DVE part [sub, bh, v-half]; PE part [half, v, bh-col]
    o_dve = nc.declare_dram_parameter("o_dve", [NSUB, D, VH], F32, isOutput=True)
    o_pe = nc.declare_dram_parameter("o_pe", [NHALF, D, 64], F32, isOutput=True)

    CSD = NC_CH * SEC * D    # 1536

    with TileContext(nc) as tc:
        with (
            tc.tile_pool(name="const", bufs=1) as const,
            tc.tile_pool(name="work", bufs=1) as work,
            tc.tile_pool(name="sdve", bufs=3) as sdve,
            tc.tile_pool(name="spe", bufs=3) as spe,
            tc.tile_pool(name="pst", bufs=2, space="PSUM") as pst,
            tc.tile_pool(name="psm", bufs=2, space="PSUM") as psm,
        ):
            # ---- input staging (scalar ring; S stream interleaved on sync) -
            t_cst = const.tile([D, (CK - 1) * CSD], F16)
            nc.sync.dma_start(t_cst[:], cst[:])
            t_xq = const.tile([D, CSD], F16)
            nc.sync.dma_start(t_xq[:], xq[:])
            t_cw = const.tile([D, CK * SEC * D], F16)
            nc.sync.dma_start(t_cw[:], cw[:])
            cw_v = t_cw[:].rearrange("p (t s d) -> p t s d", t=CK, s=SEC)
            t_fg = const.tile([D, NC_CH * D], F32)
            nc.sync.dma_start(t_fg[:], fgx[:])
            fg_v = t_fg[:].rearrange("p (c d) -> p c d", c=NC_CH)
            t_dtb = const.tile([D, D], F32)
            nc.sync.dma_start(t_dtb[:], dtb[:])
            t_misc = const.tile([D, 8], F32)
            nc.sync.dma_start(t_misc[:], misc[:])
            t_id = const.tile([D, D], F16)
            nc.sync.dma_start(t_id[:], ident[:])

            # S stream: alternate DVE sub-chunks and PE half-chunks (2.1 MB
            # each) so both engines consume the stream concurrently.
            s_tiles = []
            for i in range(NSUB):
                Sd = sdve.tile([D, VH, D], F16, name=f"Sd{i}", tag="Sd")
                nc.sync.dma_start(Sd[:], s_dve[i])
                Sp = spe.tile([D, 64, D], F16, name=f"Sp{i}", tag="Sp")
                nc.sync.dma_start(Sp[:], s_pe[i])
                s_tiles.append((Sd, Sp))

            # ---- causal conv1d single-step + silu -------------------------
            cst_v = t_cst[:].rearrange("p (t f) -> p t f", t=CK - 1)
            cst3 = cst_v.rearrange("p t (c s d) -> p t c s d", c=NC_CH, s=SEC)
            acc = work.tile([D, CSD], F16)
            tmp = work.tile([D, CSD], F16)
            acc_v = acc[:].rearrange("p (c s d) -> p c s d", c=NC_CH, s=SEC)
            tmp_v = tmp[:].rearrange("p (c s d) -> p c s d", c=NC_CH, s=SEC)
            xq_v = t_xq[:].rearrange("p (c s d) -> p c s d", c=NC_CH, s=SEC)

            def wb(j):
                return cw_v[:, j, None, :, :].to_broadcast((D, NC_CH, SEC, D))

            nc.vector.tensor_tensor(acc_v[:], cst3[:, 0], wb(0), OP.mult)
            for j in (1, 2):
                nc.vector.tensor_tensor(tmp_v[:], cst3[:, j], wb(j), OP.mult)
                nc.vector.tensor_tensor(acc[:], acc[:], tmp[:], OP.add)
            nc.vector.tensor_tensor(tmp_v[:], xq_v[:], wb(3), OP.mult)
            nc.vector.tensor_tensor(acc[:], acc[:], tmp[:], OP.add)

            x = work.tile([D, NC_CH, SEC, D], F16)
            nc.scalar.activation(x[:], acc_v[:], AF.Silu)
            xqs = x[:, :, 0, :]
            xks = x[:, :, 1, :]
            xvs = x[:, :, 2, :]

            # gate input (independent of conv; Abs groups with the silu table)
            g1 = work.tile([D, NC_CH, D], F32)
            nc.vector.tensor_tensor(
                g1[:], fg_v[:], t_dtb[:, None, :].to_broadcast((D, NC_CH, D)),
                OP.add)
            ga = work.tile([D, NC_CH, D], F32)
            nc.scalar.activation(ga[:], g1[:], AF.Abs)

            # ---- l2 norms --------------------------------------------------
            sq = work.tile([D, NC_CH, D], F16)
            nq = work.tile([D, NC_CH], F32)
            nk = work.tile([D, NC_CH], F32)
            nc.vector.tensor_tensor(sq[:], xqs, xqs, OP.mult)
            nc.vector.tensor_reduce(nq[:], sq[:], mybir.AxisListType.X, OP.add)
            nc.vector.tensor_tensor(sq[:], xks, xks, OP.mult)
            nc.vector.tensor_reduce(nk[:], sq[:], mybir.AxisListType.X, OP.add)
            nc.vector.tensor_scalar_add(nq[:], nq[:], 1e-6)
            nc.vector.tensor_scalar_add(nk[:], nk[:], 1e-6)
            rq = work.tile([D, NC_CH], F32)
            rk = work.tile([D, NC_CH], F32)
            nc.vector.reciprocal(rq[:], nq[:])
            nc.vector.reciprocal(rk[:], nk[:])
            # rsqrt = sqrt(1/x); q also gets the D^-0.5 scale folded in
            nc.scalar.activation(rq[:], rq[:], AF.Sqrt, scale=1.0 / D)
            nc.scalar.activation(rk[:], rk[:], AF.Sqrt)

            # ---- KDA gate: softplus(x) = relu(x) + ln(1+exp(-|x|)) --------
            nc.scalar.activation(ga[:], ga[:], AF.Exp, scale=-1.0)
            nc.scalar.activation(ga[:], ga[:], AF.Ln, bias=1.0)
            nc.vector.tensor_scalar_max(g1[:], g1[:], 0.0)
            nc.vector.tensor_tensor(g1[:], g1[:], ga[:], OP.add)
            nc.vector.tensor_scalar(g1[:], g1[:], t_misc[:, 0:1], None, OP.mult)
            eg = work.tile([D, NC_CH, D], F16)
            nc.scalar.activation(eg[:], g1[:], AF.Exp)
            # b = sigmoid(beta) = 1/(1+exp(-beta))
            bsig = work.tile([D, NC_CH], F32)
            nc.scalar.activation(bsig[:], t_misc[:, 1:1 + NC_CH], AF.Exp,
                                 scale=-1.0)
            nc.vector.tensor_scalar_add(bsig[:], bsig[:], 1.0)
            nc.vector.reciprocal(bsig[:], bsig[:])

            # ---- fold per-(b,h) scalars -----------------------------------
            qkr = work.tile([D, NC_CH], F32)
            nc.vector.tensor_tensor(sq[:], xqs, xks, OP.mult)
            nc.vector.tensor_reduce(qkr[:], sq[:], mybir.AxisListType.X, OP.add)
            nc.vector.tensor_tensor(qkr[:], qkr[:], rq[:], OP.mult)
            nc.vector.tensor_tensor(qkr[:], qkr[:], rk[:], OP.mult)
            cvb = work.tile([D, NC_CH], F32)      # qk*b      (for the +v term)
            nc.vector.tensor_tensor(cvb[:], qkr[:], bsig[:], OP.mult)
            mgs = work.tile([D, NC_CH], F32)      # -qk*b*rk  (fold into k)
            nc.vector.tensor_tensor(mgs[:], cvb[:], rk[:], OP.mult)
            nc.vector.tensor_scalar(mgs[:], mgs[:], -1.0, None, OP.mult)

            # ---- mg = (q*rq - qk*b*rk*k) * eg  -----------------------------
            qh = work.tile([D, NC_CH, D], F16)
            mg = work.tile([D, NC_CH, D], F16)
            for c in range(NC_CH):
                nc.vector.tensor_scalar(qh[:, c, :], xqs[:, c, :],
                                        rq[:, c:c + 1], None, OP.mult)
                nc.vector.scalar_tensor_tensor(
                    mg[:, c, :], xks[:, c, :], mgs[:, c:c + 1], qh[:, c, :],
                    OP.mult, OP.add)
            nc.vector.tensor_tensor(mg[:], mg[:], eg[:], OP.mult)

            # ---- PE-chunk prep: transpose mg and cvb*v to k/v-major --------
            mgT = []
            cvvT = []
            for c in range(PE_CH):
                tp = pst.tile([D, D], F16, name=f"tp{c}", tag="tp")
                nc.tensor.transpose(tp[:], mg[:, c, :], t_id[:])
                m16 = work.tile([D, D], F16, name=f"mgT{c}", tag="mgT")
                nc.vector.tensor_copy(m16[:], tp[:])
                mgT.append(m16)
                cvv = work.tile([D, D], F16, name=f"cvv{c}", tag="cvv")
                nc.vector.tensor_scalar(cvv[:], xvs[:, c, :], cvb[:, c:c + 1],
                                        None, OP.mult)
                tp2 = pst.tile([D, D], F16, name=f"tq{c}", tag="tp")
                nc.tensor.transpose(tp2[:], cvv[:], t_id[:])
                c32 = work.tile([D, D], F32, name=f"cvvT{c}", tag="cvvT")
                nc.vector.tensor_copy(c32[:], tp2[:])
                cvvT.append(c32)

            # ---- main loop: both engines stream their S halves -------------
            P = work.tile([D, VH, D], F16)
            for i in range(NSUB):
                Sd, Sp = s_tiles[i]
                # --- PE half-chunk i: 64 per-(b,h) stationary matmuls ------
                c, hf = divmod(i, 2)
                pso = psm.tile([D, 64], F32, name=f"pso{i}", tag="pso")
                for j in range(64):
                    col = hf * 64 + j
                    nc.tensor.matmul(pso[:, j:j + 1], Sp[:, j, :],
                                     mgT[c][:, col:col + 1],
                                     start=True, stop=True)
                ope = work.tile([D, 64], F32, name=f"ope{i}", tag="ope")
                nc.vector.tensor_tensor(
                    ope[:], pso[:], cvvT[c][:, hf * 64:hf * 64 + 64], OP.add)
                nc.scalar.dma_start(o_pe[i], ope[:])

                # --- DVE sub-chunk i: broadcast multiply + add-tree --------
                cd, vh = divmod(i, VS)
                cd += PE_CH
                nc.vector.tensor_tensor(
                    P[:], Sd[:],
                    mg[:, cd, None, :].to_broadcast((D, VH, D)), OP.mult)
                w = D // 2
                while w >= 2:
                    nc.vector.tensor_tensor(
                        P[:, :, 0:w], P[:, :, 0:w], P[:, :, w:2 * w], OP.add)
                    w //= 2
                od = work.tile([D, VH], F32, name=f"od{i}", tag="od")
                nc.vector.tensor_tensor(od[:], P[:, :, 0], P[:, :, 1], OP.add)
                nc.vector.scalar_tensor_tensor(
                    od[:], xvs[:, cd, vh * VH:(vh + 1) * VH], cvb[:, cd:cd + 1],
                    od[:], OP.mult, OP.add)
                nc.scalar.dma_start(o_dve[i], od[:])

    nc.compile()
    return nc


def _prep_inputs(mixed_qkv, forget_gate, beta, conv_state, conv_weights,
                 ssm_state, A_log, dt_bias):
    mixed_qkv = np.asarray(mixed_qkv, np.float32)
    forget_gate = np.asarray(forget_gate, np.float32)
    beta = np.asarray(beta, np.float32)
    conv_state = np.asarray(conv_state, np.float32)
    conv_weights = np.asarray(conv_weights, np.float32)
    ssm_state = np.asarray(ssm_state, np.float32)
    A_log = np.asarray(A_log, np.float32)
    dt_bias = np.asarray(dt_bias, np.float32)

    # shared (weight-like) tensors
    w = conv_weights.reshape(SEC, HV, D, CK).transpose(1, 3, 0, 2)
    cw = np.broadcast_to(w[None], (4, HV, CK, SEC, D)).reshape(D, CK * SEC * D)
    cw = np.ascontiguousarray(cw, dtype=np.float16)
    dtbp = np.ascontiguousarray(
        np.broadcast_to(dt_bias.reshape(HV, D)[None], (4, HV, D)).reshape(D, D))
    nega = np.broadcast_to((-np.exp(A_log))[None], (4, HV)).reshape(D)
    identity = np.eye(D, dtype=np.float16)

    in_maps = []
    for ci in range(NCORES):
        cs = slice(ci * BC, (ci + 1) * BC)
        ssm_c = ssm_state[cs]
        # PE chunks (batches 0..7): [half, k, (bh=64, v)]
        sp = ssm_c[0:8].reshape(PE_CH, 2, 2, HV, D, D)
        sp = sp.transpose(0, 1, 4, 2, 3, 5).reshape(NHALF, D, 64 * D)
        sp = np.ascontiguousarray(sp).astype(np.float16)
        # DVE chunks (batches 8..15): [sub, bh, v-half, k]
        sd = ssm_c[8:16].reshape(NC_CH - PE_CH, 4, HV, D, D)
        sd = sd.transpose(0, 1, 2, 4, 3).reshape(NC_CH - PE_CH, D, VS, VH, D)
        sd = sd.transpose(0, 2, 1, 3, 4).reshape(NSUB, D, VH, D)
        sd = np.ascontiguousarray(sd).astype(np.float16)
        # conv state: [16, 12288, 3] -> [p, (tap, c, sec, d)]
        cstp = conv_state[cs].reshape(NC_CH, 4, SEC, HV, D, CK - 1)
        cstp = cstp.transpose(1, 3, 5, 0, 2, 4).reshape(D, (CK - 1) * NC_CH * SEC * D)
        cstp = np.ascontiguousarray(cstp).astype(np.float16)
        xqp = mixed_qkv[cs].reshape(NC_CH, 4, SEC, HV, D)
        xqp = xqp.transpose(1, 3, 0, 2, 4).reshape(D, NC_CH * SEC * D)
        xqp = np.ascontiguousarray(xqp).astype(np.float16)
        fgp = forget_gate[cs].reshape(NC_CH, 4, HV, D)
        fgp = np.ascontiguousarray(
            fgp.transpose(1, 2, 0, 3).reshape(D, NC_CH * D))
        mi = np.zeros((D, 8), np.float32)
        mi[:, 0] = nega
        bet = beta[cs].reshape(NC_CH, 4, HV).transpose(1, 2, 0).reshape(D, NC_CH)
        mi[:, 1:1 + NC_CH] = bet
        in_maps.append({
            "s_dve": sd,
            "s_pe": sp,
            "cst": cstp,
            "xq": xqp,
            "cw": cw,
            "fgx": fgp,
            "dtb": dtbp,
            "misc": mi,
            "ident": identity,
        })
    return in_maps


def run(trace=False, **inputs):
    if "nc" not in _CACHE:
        _CACHE["nc"] = _build_nc()
    nc = _CACHE["nc"]
    in_maps = _prep_inputs(**inputs)
    res = run_bass_kernel_spmd(nc, in_maps, list(range(NCORES)), trace=trace)
    outs = []
    for ci in range(NCORES):
        r = res.results[ci]
        ope = np.asarray(r["o_pe"])   # [4, 128 v, 64 (b2,h)]
        ope = ope.reshape(PE_CH, 2, D, 2, HV).transpose(0, 1, 3, 4, 2)
        o_lo = ope.reshape(8, HV, D)
        odv = np.asarray(r["o_dve"])  # [4, 128 (b4,h), 64 vh]
        odv = odv.reshape(NC_CH - PE_CH, VS, 4, HV, VH).transpose(0, 2, 3, 1, 4)
        o_hi = odv.reshape(8, HV, D)
        outs.append(np.concatenate([o_lo, o_hi], axis=0))
    return np.concatenate(outs, axis=0), res


def kernel(**inputs) -> np.ndarray:
    out, _ = run(trace=False, **inputs)
    return out
